# revision 1
# baseline (speedup 1.0000x reference)
"""Trainium2 Bass kernel for nn_DecoderCell (LFADS-style decoder cell).

Data-parallel over 8 NeuronCores: batch 32768 -> 4096 rows/core, weights
replicated. All activations kept feature-major ([feat, batch]) on chip;
the host pre-transposes inputs / post-transposes outputs so no on-chip
transposes are needed. Matmuls run in float32r (TF32-like, 1 col/cycle).

Per 512-batch-column tile:
  controller GRU (256) -> co params -> rsample -> generator GRU (512)
  -> normalized factor projection (norm folded into weights on host).
"""

import numpy as np

B, CI, GEN, CON, CO, FAC = 32768, 128, 512, 256, 64, 128
NCORES = 8
BS = B // NCORES            # 4096 rows per core
NT = 512                    # batch columns per tile
NTILES = BS // NT           # 8
CLIP = 5.0
XIN = 2 * CI + FAC          # 384 controller input features

_CACHE = {}


def _build():
    from contextlib import ExitStack
    import concourse.bacc as bacc
    import concourse.tile as tile
    from concourse import mybir
    from concourse.bass_interp import get_hw_module

    F32R = mybir.dt.float32r
    F32 = mybir.dt.float32
    AF = mybir.ActivationFunctionType
    OP = mybir.AluOpType

    nc = bacc.Bacc("TRN2", debug=False, target_bir_lowering=False)

    # ---- DRAM I/O (per-core shard, feature-major) ----
    d_xin = nc.dram_tensor("xin", [XIN, BS], F32R, kind="ExternalInput").ap()
    d_hcon = nc.dram_tensor("hcon", [CON, BS], F32R, kind="ExternalInput").ap()
    d_hgen = nc.dram_tensor("hgen", [GEN, BS], F32R, kind="ExternalInput").ap()
    d_eps = nc.dram_tensor("epsT", [CO, BS], F32R, kind="ExternalInput").ap()
    d_wcih = nc.dram_tensor("wcih", [XIN, 3 * CON], F32R, kind="ExternalInput").ap()
    d_wchh = nc.dram_tensor("wchh", [CON, 3 * CON], F32R, kind="ExternalInput").ap()
    d_wgih = nc.dram_tensor("wgih", [CO, 3 * GEN], F32R, kind="ExternalInput").ap()
    d_wghh = nc.dram_tensor("wghh", [GEN, 3 * GEN], F32R, kind="ExternalInput").ap()
    d_wco = nc.dram_tensor("wco", [CON, 2 * CO], F32R, kind="ExternalInput").ap()
    d_wfac = nc.dram_tensor("wfac", [GEN, FAC], F32R, kind="ExternalInput").ap()
    d_bias = nc.dram_tensor("bias", [128, 20], F32, kind="ExternalInput").ap()
    d_out = nc.dram_tensor("out", [1088, BS], F32R, kind="ExternalOutput").ap()

    with tile.TileContext(nc) as tc, ExitStack() as ctx:
        wpool = ctx.enter_context(tc.tile_pool(name="w", bufs=1))
        iop = ctx.enter_context(tc.tile_pool(name="io", bufs=2))
        mid = ctx.enter_context(tc.tile_pool(name="mid", bufs=1))
        outp = ctx.enter_context(tc.tile_pool(name="out", bufs=2))
        outp1 = ctx.enter_context(tc.tile_pool(name="out1", bufs=1))
        psp = ctx.enter_context(tc.tile_pool(name="ps", bufs=8, space="PSUM"))

        # ---- weights (persistent) ----
        wcih = wpool.tile([128, 3, 3 * CON], F32R, tag="wcih")
        nc.sync.dma_start(wcih[:], d_wcih.rearrange("(k p) m -> p k m", p=128))
        wchh = wpool.tile([128, 2, 3 * CON], F32R, tag="wchh")
        nc.sync.dma_start(wchh[:], d_wchh.rearrange("(k p) m -> p k m", p=128))
        wgih = wpool.tile([CO, 3 * GEN], F32R, tag="wgih")
        nc.sync.dma_start(wgih[:], d_wgih)
        wghh = wpool.tile([128, 4, 3 * GEN], F32R, tag="wghh")
        nc.sync.dma_start(wghh[:], d_wghh.rearrange("(k p) m -> p k m", p=128))
        wco = wpool.tile([128, 2, 2 * CO], F32R, tag="wco")
        nc.sync.dma_start(wco[:], d_wco.rearrange("(k p) m -> p k m", p=128))
        wfac = wpool.tile([128, 4, FAC], F32R, tag="wfac")
        nc.sync.dma_start(wfac[:], d_wfac.rearrange("(k p) m -> p k m", p=128))
        tb = wpool.tile([128, 20], F32, tag="bias")
        nc.sync.dma_start(tb[:], d_bias)

        r_xin = d_xin.rearrange("(k p) n -> p k n", p=128)
        r_hcon = d_hcon.rearrange("(k p) n -> p k n", p=128)
        r_hgen = d_hgen.rearrange("(k p) n -> p k n", p=128)
        r_ogen = d_out[0:GEN, :].rearrange("(k p) n -> p k n", p=128)
        r_ocon = d_out[GEN:GEN + CON, :].rearrange("(k p) n -> p k n", p=128)

        for t in range(NTILES):
            cs = slice(t * NT, (t + 1) * NT)

            # ---- loads ----
            txin = iop.tile([128, 3, NT], F32R, tag="xin")
            nc.sync.dma_start(txin[:], r_xin[:, :, cs])
            thcon = iop.tile([128, 2, NT], F32R, tag="hcon")
            nc.sync.dma_start(thcon[:], r_hcon[:, :, cs])
            thgen = iop.tile([128, 4, NT], F32R, tag="hgen")
            nc.sync.dma_start(thgen[:], r_hgen[:, :, cs])
            teps = iop.tile([CO, NT], F32R, tag="eps")
            nc.sync.dma_start(teps[:], d_eps[:, cs])

            # ---- controller GRU ----
            # z,r pre-activations: xin @ Wih_zr + hcon @ Whh_zr  (4 banks)
            zc = mid.tile([128, 2, NT], F32R, tag="zc")
            rc = mid.tile([128, 2, NT], F32R, tag="rc")
            pn_list = []
            for mb in range(4):
                p = psp.tile([128, NT], F32, tag="ps")
                ms = slice(mb * 128, (mb + 1) * 128)
                for k in range(3):
                    nc.tensor.matmul(p[:], wcih[:, k, ms], txin[:, k, :],
                                     start=(k == 0), stop=False)
                for k in range(2):
                    nc.tensor.matmul(p[:], wchh[:, k, ms], thcon[:, k, :],
                                     start=False, stop=(k == 1))
                dst = zc if mb < 2 else rc
                nc.scalar.activation(dst[:, mb % 2, :], p[:], AF.Sigmoid,
                                     bias=tb[:, mb:mb + 1], scale=1.0)
            # n pre-activation: xn part first (r*h part accumulated later)
            for mb in range(2):
                p = psp.tile([128, NT], F32, tag="ps")
                ms = slice(2 * CON + mb * 128, 2 * CON + (mb + 1) * 128)
                for k in range(3):
                    nc.tensor.matmul(p[:], wcih[:, k, ms], txin[:, k, :],
                                     start=(k == 0), stop=False,
                                     skip_group_check=True)
                pn_list.append(p)
            # rh = r * hcon
            rhc = mid.tile([128, 2, NT], F32R, tag="rhc")
            nc.vector.tensor_tensor(rhc[:], rc[:], thcon[:], OP.mult)
            ncn = mid.tile([128, 2, NT], F32R, tag="ncn")
            for mb in range(2):
                p = pn_list[mb]
                ms = slice(2 * CON + mb * 128, 2 * CON + (mb + 1) * 128)
                for k in range(2):
                    nc.tensor.matmul(p[:], wchh[:, k, ms], rhc[:, k, :],
                                     start=False, stop=(k == 1),
                                     skip_group_check=True)
                nc.scalar.activation(ncn[:, mb, :], p[:], AF.Tanh,
                                     bias=tb[:, 4 + mb:5 + mb], scale=1.0)
            # h' = n + z*(h-n), clip
            dc = mid.tile([128, 2, NT], F32R, tag="dc")
            nc.vector.tensor_tensor(dc[:], thcon[:], ncn[:], OP.subtract)
            zdc = mid.tile([128, 2, NT], F32R, tag="zdc")
            nc.vector.tensor_tensor(zdc[:], zc[:], dc[:], OP.mult)
            hc = mid.tile([128, 2, NT], F32R, tag="hc")
            nc.vector.tensor_tensor(hc[:], ncn[:], zdc[:], OP.add)
            tcs = outp.tile([128, 2, NT], F32R, tag="cs")
            nc.gpsimd.tensor_scalar(tcs[:], hc[:], CLIP, -CLIP, OP.min, OP.max)
            nc.sync.dma_start(r_ocon[:, :, cs], tcs[:])

            # ---- co params + rsample ----
            pm = psp.tile([CO, NT], F32, tag="ps")
            plv = psp.tile([CO, NT], F32, tag="ps")
            for k in range(2):
                nc.tensor.matmul(pm[:], wco[:, k, 0:CO], tcs[:, k, :],
                                 start=(k == 0), stop=(k == 1))
            for k in range(2):
                nc.tensor.matmul(plv[:], wco[:, k, CO:2 * CO], tcs[:, k, :],
                                 start=(k == 0), stop=(k == 1))
            tmean = outp1.tile([CO, NT], F32R, tag="mean")
            nc.scalar.activation(tmean[:], pm[:], AF.Identity,
                                 bias=tb[0:CO, 18:19], scale=1.0)
            tstd = outp1.tile([CO, NT], F32R, tag="std")
            nc.scalar.activation(tstd[:], plv[:], AF.Exp,
                                 bias=tb[0:CO, 19:20], scale=0.5)
            nc.sync.dma_start(d_out[GEN + CON:GEN + CON + CO, cs], tmean[:])
            nc.sync.dma_start(d_out[GEN + CON + CO:GEN + CON + 2 * CO, cs], tstd[:])
            tse = mid.tile([CO, NT], F32R, tag="se")
            nc.gpsimd.tensor_tensor(tse[:], tstd[:], teps[:], OP.mult)
            tgi = outp1.tile([CO, NT], F32R, tag="gi")
            nc.gpsimd.tensor_tensor(tgi[:], tmean[:], tse[:], OP.add)
            nc.sync.dma_start(d_out[GEN + CON + 2 * CO:GEN + CON + 3 * CO, cs],
                              tgi[:])

            # ---- generator GRU ----
            zg = mid.tile([128, 4, NT], F32R, tag="zg")
            rg = mid.tile([128, 4, NT], F32R, tag="rg")
            png = []
            for mb in range(8):
                p = psp.tile([128, NT], F32, tag="ps")
                ms = slice(mb * 128, (mb + 1) * 128)
                for k in range(4):
                    nc.tensor.matmul(p[:], wghh[:, k, ms], thgen[:, k, :],
                                     start=(k == 0), stop=False,
                                     skip_group_check=True)
                nc.tensor.matmul(p[:], wgih[:, ms], tgi[:],
                                 start=False, stop=True, skip_group_check=True)
                dst = zg if mb < 4 else rg
                nc.scalar.activation(dst[:, mb % 4, :], p[:], AF.Sigmoid,
                                     bias=tb[:, 6 + mb:7 + mb], scale=1.0)
            rhg = mid.tile([128, 4, NT], F32R, tag="rhg")
            nc.vector.tensor_tensor(rhg[:], rg[:], thgen[:], OP.mult)
            ngn = mid.tile([128, 4, NT], F32R, tag="ngn")
            for mb in range(4):
                p = psp.tile([128, NT], F32, tag="ps")
                ms = slice(2 * GEN + mb * 128, 2 * GEN + (mb + 1) * 128)
                nc.tensor.matmul(p[:], wgih[:, ms], tgi[:],
                                 start=True, stop=False, skip_group_check=True)
                for k in range(4):
                    nc.tensor.matmul(p[:], wghh[:, k, ms], rhg[:, k, :],
                                     start=False, stop=(k == 3),
                                     skip_group_check=True)
                nc.scalar.activation(ngn[:, mb, :], p[:], AF.Tanh,
                                     bias=tb[:, 14 + mb:15 + mb], scale=1.0)
            dg = mid.tile([128, 4, NT], F32R, tag="dg")
            nc.vector.tensor_tensor(dg[:], thgen[:], ngn[:], OP.subtract)
            zdg = mid.tile([128, 4, NT], F32R, tag="zdg")
            nc.vector.tensor_tensor(zdg[:], zg[:], dg[:], OP.mult)
            hg = mid.tile([128, 4, NT], F32R, tag="hg")
            nc.vector.tensor_tensor(hg[:], ngn[:], zdg[:], OP.add)
            tgs = outp.tile([128, 4, NT], F32R, tag="gs")
            nc.gpsimd.tensor_scalar(tgs[:], hg[:], CLIP, -CLIP, OP.min, OP.max)
            nc.sync.dma_start(r_ogen[:, :, cs], tgs[:])

            # ---- factors ----
            pf = psp.tile([FAC, NT], F32, tag="ps")
            for k in range(4):
                nc.tensor.matmul(pf[:], wfac[:, k, :], tgs[:, k, :],
                                 start=(k == 0), stop=(k == 3))
            tfc = outp1.tile([FAC, NT], F32R, tag="fc")
            nc.scalar.activation(tfc[:], pf[:], AF.Copy)
            nc.sync.dma_start(d_out[GEN + CON + 3 * CO:1088, cs], tfc[:])

    nc.compile()
    nc.m = get_hw_module(nc.m)
    return nc


def _prep_inputs(inputs):
    inp = np.asarray(inputs["input"], dtype=np.float32)
    h0 = np.asarray(inputs["h_0"], dtype=np.float32)
    eps = np.asarray(inputs["eps"], dtype=np.float32)

    f32 = np.float32
    wcih = np.ascontiguousarray(np.asarray(inputs["con_Wih"], f32).T)
    wchh = np.ascontiguousarray(np.asarray(inputs["con_Whh"], f32).T)
    wgih = np.ascontiguousarray(np.asarray(inputs["gen_Wih"], f32).T)
    wghh = np.ascontiguousarray(np.asarray(inputs["gen_Whh"], f32).T)
    wco = np.ascontiguousarray(np.asarray(inputs["co_W"], f32).T)
    fw = np.asarray(inputs["fac_W"], f32)
    norm = np.sqrt((fw.astype(np.float64) ** 2).sum(axis=1, keepdims=True))
    nw = (fw / np.maximum(norm, 1e-12)).astype(f32)
    wfac = np.ascontiguousarray(nw.T)

    bias = np.zeros((128, 20), dtype=f32)
    cb = (np.asarray(inputs["con_bih"], f32) + np.asarray(inputs["con_bhh"], f32))
    gb = (np.asarray(inputs["gen_bih"], f32) + np.asarray(inputs["gen_bhh"], f32))
    cob = np.asarray(inputs["co_b"], f32)
    for mb in range(2):
        bias[:, mb] = cb[0 * CON + mb * 128:0 * CON + (mb + 1) * 128]
        bias[:, 2 + mb] = cb[1 * CON + mb * 128:1 * CON + (mb + 1) * 128]
        bias[:, 4 + mb] = cb[2 * CON + mb * 128:2 * CON + (mb + 1) * 128]
    for mb in range(4):
        bias[:, 6 + mb] = gb[0 * GEN + mb * 128:0 * GEN + (mb + 1) * 128]
        bias[:, 10 + mb] = gb[1 * GEN + mb * 128:1 * GEN + (mb + 1) * 128]
        bias[:, 14 + mb] = gb[2 * GEN + mb * 128:2 * GEN + (mb + 1) * 128]
    bias[0:CO, 18] = cob[0:CO]
    bias[0:CO, 19] = 0.5 * cob[CO:2 * CO]

    shared = {"wcih": wcih, "wchh": wchh, "wgih": wgih, "wghh": wghh,
              "wco": wco, "wfac": wfac, "bias": bias}
    in_maps = []
    for c in range(NCORES):
        r = slice(c * BS, (c + 1) * BS)
        m = dict(shared)
        m["xin"] = np.ascontiguousarray(
            np.concatenate([inp[r], h0[r, GEN + CON + 3 * CO:1088]], axis=1).T)
        m["hcon"] = np.ascontiguousarray(h0[r, GEN:GEN + CON].T)
        m["hgen"] = np.ascontiguousarray(h0[r, 0:GEN].T)
        m["epsT"] = np.ascontiguousarray(eps[r].T)
        in_maps.append(m)
    return in_maps


def _run(in_maps, trace=False, **kw):
    from concourse.bass_utils import run_bass_kernel_spmd
    if "nc" not in _CACHE:
        _CACHE["nc"] = _build()
    return run_bass_kernel_spmd(_CACHE["nc"], in_maps,
                                core_ids=list(range(NCORES)), trace=trace, **kw)


def kernel(**inputs):
    in_maps = _prep_inputs(inputs)
    res = _run(in_maps)
    out = np.empty((B, 1088), dtype=np.float32)
    for c in range(NCORES):
        out[c * BS:(c + 1) * BS] = res.results[c]["out"].T
    return out
